# revision 21
# baseline (speedup 1.0000x reference)
"""ConcatCritic pair-grid MLP on 8 TRN2 NeuronCores.

Computes out[i, j] = f(x[i], y[j]) where f is a 3-hidden-layer MLP over the
concatenated pair, decomposed so the first layer is two small projections
summed by broadcast (no [B, B, A+B] concat tensor).

Sharding: the B^2 pair grid is split row-wise (x batch) across 8 cores;
y and all MLP parameters are replicated. Each core produces a [B/8, B]
score tile; the host concatenates them. b3 (a scalar) is added on the host.

Device layout: activations live transposed as [hid-on-partitions, pairs-on-
free] so every layer matmul is lhsT=W_block [128(k),128(m)], rhs=hT
[128(k), 512(pairs)] accumulating over 4 k-blocks into PSUM. Matmul operands
are float32r (fp22 multiply at full PE rate, fp32 accumulate). PSUM->SBUF
relu+bias drains are split between ScalarE and VectorE to keep both under
the TensorE span.
"""

import numpy as np

import concourse.bass as bass
import concourse.mybir as mybir
from concourse import bacc
from concourse.bass_utils import run_bass_kernel_spmd
from concourse.tile import TileContext

B = 256
A_DIM = 128
HID = 512
N_CORES = 8
ROWS = B // N_CORES  # 32 x-rows per core
KB = HID // 128  # 4 k-blocks of 128
PAIR_TILE = 512  # pairs per matmul tile = 2 x-rows x 256 y-rows
ROWS_PER_TILE = PAIR_TILE // B  # 2
N_TILES = ROWS // ROWS_PER_TILE  # 16

F32 = mybir.dt.float32
F32R = mybir.dt.float32r

_CACHE = {}


def _build_nc():
    nc = bacc.Bacc()

    xT = nc.declare_dram_parameter("xT", [A_DIM, ROWS], F32R, isOutput=False)
    yT = nc.declare_dram_parameter("yT", [A_DIM, B], F32R, isOutput=False)
    Wx = nc.declare_dram_parameter("Wx", [A_DIM, HID], F32R, isOutput=False)
    Wy = nc.declare_dram_parameter("Wy", [A_DIM, HID], F32R, isOutput=False)
    W1 = nc.declare_dram_parameter("W1", [HID, HID], F32R, isOutput=False)
    W2 = nc.declare_dram_parameter("W2", [HID, HID], F32R, isOutput=False)
    W3 = nc.declare_dram_parameter("W3", [HID, 1], F32R, isOutput=False)
    b0r = nc.declare_dram_parameter("b0r", [128, KB], F32, isOutput=False)
    b1r = nc.declare_dram_parameter("b1r", [128, KB], F32, isOutput=False)
    b2r = nc.declare_dram_parameter("b2r", [128, KB], F32, isOutput=False)
    out = nc.declare_dram_parameter("out", [1, ROWS * B], F32, isOutput=True)

    relu = mybir.ActivationFunctionType.Relu

    with TileContext(nc) as tc:
        with (
            tc.tile_pool(name="const", bufs=1) as const,
            tc.tile_pool(name="work", bufs=3) as work,
            tc.tile_pool(name="sc_pool", bufs=4) as sc_pool,
            tc.tile_pool(name="psum", bufs=6, space="PSUM") as psum,
            tc.tile_pool(name="psum_s", bufs=2, space="PSUM") as psum_s,
        ):
            # ---- load replicated constants -------------------------------
            xT_sb = const.tile([A_DIM, ROWS], F32R)
            yT_sb = const.tile([A_DIM, B], F32R)
            Wx_sb = const.tile([A_DIM, HID], F32R)
            Wy_sb = const.tile([A_DIM, HID], F32R)
            b0_sb = const.tile([128, KB], F32)
            b1_sb = const.tile([128, KB], F32)
            b2_sb = const.tile([128, KB], F32)
            W1_sb = const.tile([128, KB, HID], F32R)
            W2_sb = const.tile([128, KB, HID], F32R)
            W3_sb = const.tile([128, KB, 1], F32R)

            nc.sync.dma_start(xT_sb[:], xT[:, :])
            nc.sync.dma_start(Wx_sb[:], Wx[:, :])
            nc.sync.dma_start(yT_sb[:], yT[:, :])
            nc.sync.dma_start(Wy_sb[:], Wy[:, :])
            nc.sync.dma_start(b0_sb[:], b0r[:, :])
            # W1 chunks before anything L2 needs: tile-0 layer-1 k-group
            # matmuls gate on W1 k-block arrival.
            w1_r = W1[:, :].rearrange("(k p) n -> p k n", p=128)
            w2_r = W2[:, :].rearrange("(k p) n -> p k n", p=128)
            for k in range(KB):
                nc.sync.dma_start(W1_sb[:, k], w1_r[:, k])
            nc.sync.dma_start(b1_sb[:], b1r[:, :])
            for k in range(KB):
                nc.sync.dma_start(W2_sb[:, k], w2_r[:, k])
            nc.sync.dma_start(b2_sb[:], b2r[:, :])
            nc.sync.dma_start(W3_sb[:], W3[:, :].rearrange("(k p) n -> p k n", p=128))

            # ---- input projections --------------------------------------
            # bxT[p, m, i] = (x @ Wx)^T[m*128+p, i] + b0[m*128+p]
            # hx/hy interleaved per block m and drains split DVE/ACT so the
            # first pair-tile's layer-0 (DVE) and layer-1 (PE) start early.
            bxT = const.tile([128, KB, ROWS], F32)
            hyT = const.tile([128, KB, B], F32)
            for m in range(KB):
                sl = slice(m * 128, (m + 1) * 128)
                ph = psum.tile([128, PAIR_TILE], F32, tag="ps", name="ph")[:, :ROWS]
                nc.tensor.matmul(ph, Wx_sb[:, sl], xT_sb[:], start=True, stop=True)
                nc.vector.tensor_scalar_add(bxT[:, m], ph, b0_sb[:, m : m + 1])
                ph2 = psum.tile([128, PAIR_TILE], F32, tag="ps", name="ph2")[:, :B]
                nc.tensor.matmul(ph2, Wy_sb[:, sl], yT_sb[:], start=True, stop=True)
                nc.scalar.copy(out=hyT[:, m], in_=ph2)

            # ---- main pair-tile loop ------------------------------------
            for t in range(N_TILES):
                i0 = t * ROWS_PER_TILE
                # layer 0 on DVE (SBUF->SBUF is cheap there):
                # h0T[p, k, a*256+j] = relu(hyT[p,k,j] + bxT[p,k,i0+a])
                h0T = work.tile([128, KB, PAIR_TILE], F32R, tag="h0")
                for k in range(KB):
                    for a in range(ROWS_PER_TILE):
                        nc.vector.tensor_scalar(
                            h0T[:, k, a * B : (a + 1) * B],
                            hyT[:, k],
                            bxT[:, k, i0 + a : i0 + a + 1],
                            0.0,
                            mybir.AluOpType.add,
                            mybir.AluOpType.max,
                        )
                # layers 1 and 2; PSUM drains (relu+bias) split 5:3 between
                # ScalarE and VectorE so both stay under the TensorE span.
                hin = h0T
                for layer, (W_sb, b_sb) in enumerate(((W1_sb, b1_sb), (W2_sb, b2_sb))):
                    hout = work.tile([128, KB, PAIR_TILE], F32R, tag=f"h{layer + 1}")
                    for m in range(KB):
                        pt = psum.tile([128, PAIR_TILE], F32, tag="ps", name="pt")
                        for k in range(KB):
                            nc.tensor.matmul(
                                pt,
                                W_sb[:, k, m * 128 : (m + 1) * 128],
                                hin[:, k],
                                start=(k == 0),
                                stop=(k == KB - 1),
                            )
                        on_act = (m % 2 == 0) if layer == 0 else (m != 3)
                        if on_act:
                            nc.scalar.activation(
                                hout[:, m],
                                pt,
                                relu,
                                bias=b_sb[:, m : m + 1],
                                scale=1.0,
                            )
                        else:
                            nc.vector.tensor_scalar(
                                hout[:, m],
                                pt,
                                b_sb[:, m : m + 1],
                                0.0,
                                mybir.AluOpType.add,
                                mybir.AluOpType.max,
                            )
                    hin = hout
                # layer 3: [1, 512] scores for this tile (b3 added on host)
                ps = psum_s.tile([128, PAIR_TILE], F32, tag="sc", name="ps")[:1]
                for k in range(KB):
                    nc.tensor.matmul(
                        ps,
                        W3_sb[:, k],
                        hin[:, k],
                        start=(k == 0),
                        stop=(k == KB - 1),
                    )
                sc_sb = sc_pool.tile([1, PAIR_TILE], F32, tag="sc_sb")
                nc.scalar.copy(out=sc_sb[:], in_=ps)
                nc.sync.dma_start(
                    out[:, t * PAIR_TILE : (t + 1) * PAIR_TILE], sc_sb[:]
                )

    nc.compile()
    return nc


def _get_nc():
    if "nc" not in _CACHE:
        _CACHE["nc"] = _build_nc()
    return _CACHE["nc"]


def _prep_in_maps(inputs):
    f = lambda a: np.ascontiguousarray(np.asarray(a), dtype=np.float32)
    x, y = f(inputs["x"]), f(inputs["y"])
    shared = {
        "yT": f(y.T),
        "Wx": f(inputs["Wx"]),
        "Wy": f(inputs["Wy"]),
        "W1": f(inputs["W1"]),
        "W2": f(inputs["W2"]),
        "W3": f(inputs["W3"]),
        "b0r": f(np.asarray(inputs["b0"]).reshape(KB, 128).T),
        "b1r": f(np.asarray(inputs["b1"]).reshape(KB, 128).T),
        "b2r": f(np.asarray(inputs["b2"]).reshape(KB, 128).T),
    }
    in_maps = []
    for m in range(N_CORES):
        im = dict(shared)
        im["xT"] = f(x[m * ROWS : (m + 1) * ROWS].T)
        in_maps.append(im)
    return in_maps


def run(trace=False, **inputs):
    nc = _get_nc()
    in_maps = _prep_in_maps(inputs)
    res = run_bass_kernel_spmd(nc, in_maps, core_ids=list(range(N_CORES)), trace=trace)
    b3 = np.float32(np.asarray(inputs["b3"]).reshape(-1)[0])
    blocks = [r["out"].reshape(ROWS, B) + b3 for r in res.results]
    return np.concatenate(blocks, axis=0).astype(np.float32), res


def kernel(**inputs):
    out, _ = run(trace=False, **inputs)
    return out
